# Initial kernel scaffold
#
"""Chamfer distance (K=1 squared-euclidean NN, both directions) on 8
Trainium2 NeuronCores.

Sharding: 8 independent work units = 4 batches x 2 directions; one unit per
core (SPMD — same program, different inputs). Per unit: queries Q[8192,3]
vs keys K[8192,3].

Device algorithm per unit:
  u[p,q] = 2*q_p.k_q - ||q_p||^2 - ||k_q||^2 = -(d^2)     via K=5 matmul
    with augmented operands lhsT = [2qx,2qy,2qz,||q||^2,1],
                            rhs  = [kx,ky,kz,-1,-||k||^2].
  * TensorE: K=5/M=128/N=512 matmuls fill [128,2048] 4-bank PSUM supertiles
    (two supertiles ping-pong). Per-block weights are staged to a fixed SBUF
    slot by DMA (walrus forbids register offsets in matmul weights).
  * VectorE: ONE 1x-rate pass — a running prefix-max scan
    (tensor_tensor_scan, op=max, carry chained across supertiles) PSUM ->
    SBUF. M = last scan element = exact max of u; cham = -M (negated on
    host).
  * ScalarE: argmax (first occurrence) via a counting trick:
    sign(M - scan_q) is +1 exactly for q < q_first and 0 after, so one
    activation(Sign, scale=-1, bias=M, accum_out) yields idx directly as an
    exact fp32 integer (int-cast on host).
  * Sync engine (SP): stages weights and streams per-block results (M, idx)
    to DRAM with dynamic-offset DMAs.

The whole program runs inside hardware Fori loops (pair-of-blocks bodies so
buffers alternate with static APs): this environment charges a large fixed
cost per *unique* instruction, so the program is ~130 instructions
re-executed via branches. Cross-engine sync is credit-based: semaphores +
per-waiter credit registers (wait_ge takes a register), with
`nop().then_inc(sem, n)` priming standing in for negative initial credits.
"""

from contextlib import ExitStack

import numpy as np

import concourse.bass as bass
import concourse.mybir as mybir
from concourse.bass import ds
from concourse.bass_utils import run_bass_kernel_spmd

F32 = mybir.dt.float32
NEG_BIG = -3.0e38

N_BATCH = 4
NPTS = 8192
N_CORES = 8


def build_chamfer_bass(P1=NPTS, P2=NPTS, repeat=1):
    """Single-core Bass program (SPMD across cores)."""
    sup = 2048                       # keys per supertile (4 PSUM banks)
    assert P1 % 256 == 0 and P2 % (2 * sup) == 0
    qb = P1 // 128                   # query blocks
    pairs = qb // 2                  # 2 blocks per loop iteration
    nsup = P2 // sup                 # supertiles per block (even)
    ntile = sup // 512
    assert nsup % 2 == 0

    nc = bass.Bass()
    qka = nc.dram_tensor("qka", [5, P1 + P2], F32, kind="ExternalInput")
    mval = nc.dram_tensor("mval", [128, qb], F32, kind="ExternalOutput")
    idxf = nc.dram_tensor("idxf", [128, qb], F32, kind="ExternalOutput")

    with ExitStack() as ctx:
        ec = ctx.enter_context
        keys_sb = ec(nc.sbuf_tensor([5, P2], F32))
        wstage = ec(nc.sbuf_tensor([5, 256], F32))  # A: 0:128, B: 128:256
        dummy = ec(nc.sbuf_tensor([128, sup], F32))
        scan_a = ec(nc.sbuf_tensor([128, P2], F32))
        scan_b = ec(nc.sbuf_tensor([128, P2], F32))
        junk = ec(nc.sbuf_tensor([128, P2], mybir.dt.bfloat16))
        m1 = ec(nc.sbuf_tensor([128, 2], F32))      # per-half M
        a1 = ec(nc.sbuf_tensor([128, 2], F32))      # per-half idx (float)
        ps_a = ec(nc.psum_tensor([128, sup], F32))
        ps_b = ec(nc.psum_tensor([128, sup], F32))
        s_dma = ec(nc.semaphore("s_dma"))
        s_stage = ec(nc.semaphore("s_stage"))
        s_mm = ec(nc.semaphore("s_mm"))
        s_scan = ec(nc.semaphore("s_scan"))
        s_act = ec(nc.semaphore("s_act"))
        s_ext = ec(nc.semaphore("s_ext"))
        s_out = ec(nc.semaphore("s_out"))
        block = ec(nc.Block())
        ps = [ps_a, ps_b]
        scans = [scan_a, scan_b]

        @block.sync
        def _(sync):
            sync.dma_start(out=keys_sb[:, :], in_=qka[:, P1:P1 + P2]
                           ).then_inc(s_dma, 16)
            sync.dma_start(
                out=dummy[:, :],
                in_=bass.AP(tensor=qka, offset=0, ap=[[0, 128], [1, sup]]),
            ).then_inc(s_dma, 16)
            qoff = sync.alloc_register("qoff")
            mreg = sync.alloc_register("mreg")
            r_pe = sync.alloc_register("r_pe")
            r_sg = sync.alloc_register("r_sg")
            r_sa = sync.alloc_register("r_sa")
            sync.reg_mov(r_pe, 0)
            sync.reg_mov(r_sg, 0)
            sync.reg_mov(r_sa, 2)

            def stage_pair():
                qv = sync.snap(qoff, min_val=0, max_val=P1)
                sync.dma_start(out=wstage[:, 0:128],
                               in_=qka[:, ds(qv, 128)]).then_inc(s_stage, 16)
                sync.dma_start(out=wstage[:, 128:256],
                               in_=qka[:, ds(qv + 128, 128)]
                               ).then_inc(s_stage, 16)
                sync.reg_add(qoff, qoff, 256)

            def outs_block(half):
                # results of global block (mreg) are in slot `half`
                sync.reg_add(r_sa, r_sa, 1)
                sync.wait_ge(s_act, r_sa)
                mv = sync.snap(mreg, min_val=0, max_val=qb - 1)
                with nc.allow_non_contiguous_dma(
                        reason="128 scattered 4B column writes per block"):
                    sync.dma_start(out=mval[:, ds(mv, 1)],
                                   in_=m1[:, half:half + 1]
                                   ).then_inc(s_out, 16)
                    sync.dma_start(out=idxf[:, ds(mv, 1)],
                                   in_=a1[:, half:half + 1]
                                   ).then_inc(s_out, 16)
                sync.reg_add(mreg, mreg, 1)

            with sync.Fori(0, repeat, 1):
                sync.reg_mov(qoff, 0)
                sync.reg_mov(mreg, 0)
                # previous rep's staging fully drained (wstage WAW)
                sync.wait_ge(s_stage, r_sg)
                sync.reg_add(r_sg, r_sg, 32 * (pairs + 1))
                stage_pair()                      # pair 0
                if pairs > 1:
                    with sync.Fori(0, pairs - 1, 1):
                        # stage pair i+1 once pair i's matmuls are done
                        sync.reg_add(r_pe, r_pe, 2 * nsup)
                        sync.wait_ge(s_mm, r_pe)
                        stage_pair()
                        # stream out pair i's results (signs lag matmuls
                        # by about a block, so they don't stall staging)
                        outs_block(0)
                        outs_block(1)
                sync.reg_add(r_pe, r_pe, 2 * nsup)  # last pair's matmuls
                sync.wait_ge(s_mm, r_pe)
                stage_pair()                      # garbage pre-stage
                outs_block(0)
                outs_block(1)
            sync.wait_ge(s_out, 64 * pairs * repeat)
            sync.wait_ge(s_dma, 32)

        @block.tensor
        def _(tensor):
            tensor.wait_ge(s_dma, 32)
            r_stage = tensor.alloc_register("r_stage")
            r_scan = tensor.alloc_register("r_scan")
            tensor.reg_mov(r_stage, 0)
            tensor.reg_mov(r_scan, 0)
            with tensor.Fori(0, repeat, 1):
                with tensor.Fori(0, pairs, 1):
                    # pair staged (both DMAs; completion order arbitrary)
                    tensor.reg_add(r_stage, r_stage, 32)
                    tensor.wait_ge(s_stage, r_stage)
                    for half in range(2):
                        lhsT = wstage[:, 128 * half:128 * half + 128]
                        for s in range(nsup):
                            # psum slot free: scan of its previous use done
                            # (s_scan primed with 2 credits)
                            tensor.reg_add(r_scan, r_scan, 1)
                            tensor.wait_ge(s_scan, r_scan)
                            last = None
                            for t in range(ntile):
                                last = nc.tensor.matmul(
                                    ps[s % 2][:, 512 * t:512 * (t + 1)],
                                    lhsT,
                                    keys_sb[:, sup * s + 512 * t:
                                            sup * s + 512 * (t + 1)],
                                    start=True, stop=True,
                                )
                            last.then_inc(s_mm, 1)
                # swallow the end-of-rep garbage staging batch's credits
                tensor.reg_add(r_stage, r_stage, 32)

        @block.vector
        def _(vector):
            vector.wait_ge(s_dma, 32)
            vector.nop().then_inc(s_scan, 2)      # psum-reuse priming
            r_mm = vector.alloc_register("r_mm")
            r_act = vector.alloc_register("r_act")
            r_self = vector.alloc_register("r_self")
            vector.reg_mov(r_mm, 0)
            vector.reg_mov(r_act, 0)
            vector.reg_mov(r_self, 2)
            with vector.Fori(0, pairs * repeat, 1):
                for half in range(2):
                    sb = scans[half]
                    # scanbuf slot free: sign of its previous user done
                    # (s_act primed with 2 credits)
                    vector.reg_add(r_act, r_act, 1)
                    vector.wait_ge(s_act, r_act)
                    for s in range(nsup):
                        vector.reg_add(r_mm, r_mm, 1)
                        vector.wait_ge(s_mm, r_mm)
                        if s > 0:
                            # carry element committed
                            vector.wait_ge(s_scan, r_self)
                        init = (NEG_BIG if s == 0
                                else sb[:, sup * s - 1:sup * s])
                        nc.vector.tensor_tensor_scan(
                            out=sb[:, sup * s:sup * (s + 1)],
                            data0=ps[s % 2][:, :],
                            data1=dummy[:, :],
                            initial=init,
                            op0=mybir.AluOpType.max,
                            op1=mybir.AluOpType.bypass,
                        ).then_inc(s_scan, 1)
                        vector.reg_add(r_self, r_self, 1)

        @block.scalar
        def _(scalar):
            scalar.nop().then_inc(s_act, 2)       # scanbuf-reuse priming
            r_scan = scalar.alloc_register("r_scan")
            r_ext = scalar.alloc_register("r_ext")
            r_jw = scalar.alloc_register("r_jw")
            r_so = scalar.alloc_register("r_so")
            scalar.reg_mov(r_scan, 2)
            scalar.reg_mov(r_ext, 0)
            scalar.reg_mov(r_jw, 1)
            scalar.reg_mov(r_so, 0)
            with scalar.Fori(0, pairs * repeat, 1):
                # sync's copy-outs of the previous pair (this pair's slots'
                # previous users) must be done; their completion order is
                # arbitrary so wait at pair granularity
                scalar.wait_ge(s_out, r_so)
                scalar.reg_add(r_so, r_so, 64)
                for half in range(2):
                    sb = scans[half]
                    scalar.reg_add(r_scan, r_scan, nsup)
                    scalar.wait_ge(s_scan, r_scan)
                    nc.scalar.activation(
                        out=m1[:, half:half + 1], in_=sb[:, P2 - 1:P2],
                        func=mybir.ActivationFunctionType.Copy, scale=1.0,
                    ).then_inc(s_ext, 1)
                    # extract committed (bias RAW)
                    scalar.reg_add(r_ext, r_ext, 1)
                    scalar.wait_ge(s_ext, r_ext)
                    # previous sign committed (junk WAW; s_act primed +2)
                    scalar.reg_add(r_jw, r_jw, 1)
                    scalar.wait_ge(s_act, r_jw)
                    # sign(M - scan_q): +1 before first argmax, 0 after;
                    # accum = first-occurrence argmax as exact fp32 int
                    nc.scalar.activation(
                        out=junk[:, :], in_=sb[:, :],
                        func=mybir.ActivationFunctionType.Sign,
                        bias=m1[:, half:half + 1], scale=-1.0,
                        accum_out=a1[:, half:half + 1],
                    ).then_inc(s_act, 1)

    return nc


def make_unit_inputs(q, k):
    """Host-side augmentation for one (query cloud, key cloud) unit."""
    q = np.ascontiguousarray(q, np.float32)
    k = np.ascontiguousarray(k, np.float32)
    p1, p2 = q.shape[0], k.shape[0]
    qka = np.empty((5, p1 + p2), np.float32)
    qka[0:3, :p1] = 2.0 * q.T
    qka[3, :p1] = (q * q).sum(-1, dtype=np.float32)
    qka[4, :p1] = 1.0
    qka[0:3, p1:] = k.T
    qka[3, p1:] = -1.0
    qka[4, p1:] = -((k * k).sum(-1, dtype=np.float32))
    return {"qka": qka}


_BUILT = {}


def _built_nc():
    if "nc" not in _BUILT:
        _BUILT["nc"] = build_chamfer_bass()
    return _BUILT["nc"]


def kernel(x, y, _collect_results=None):
    """Full-input entry point. x, y: (4, 8192, 3) float32.

    Returns (cham_x, cham_y, idx_x, idx_y) matching reference()."""
    x = np.asarray(x, np.float32)
    y = np.asarray(y, np.float32)
    n = x.shape[0]
    units = []
    in_maps = []
    for b in range(n):
        for d in range(2):
            q, k = (x[b], y[b]) if d == 0 else (y[b], x[b])
            in_maps.append(make_unit_inputs(q, k))
            units.append((b, d))
    nc = _built_nc()
    res = run_bass_kernel_spmd(nc, in_maps, core_ids=list(range(N_CORES)))
    if _collect_results is not None:
        _collect_results.append(res)
    cham_x = np.empty((n, x.shape[1]), np.float32)
    cham_y = np.empty((n, y.shape[1]), np.float32)
    idx_x = np.empty((n, x.shape[1]), np.int32)
    idx_y = np.empty((n, y.shape[1]), np.int32)
    for (b, d), r in zip(units, res.results):
        chamv = (-np.asarray(r["mval"])).T.reshape(-1)
        idxv = np.asarray(r["idxf"]).T.reshape(-1).astype(np.int32)
        if d == 0:
            cham_x[b], idx_x[b] = chamv, idxv
        else:
            cham_y[b], idx_y[b] = chamv, idxv
    return cham_x, cham_y, idx_x, idx_y



# revision 17
# speedup vs baseline: 3.0077x; 3.0077x over previous
"""Chamfer distance (K=1 squared-euclidean NN, both directions) on 8
Trainium2 NeuronCores.

Sharding: 8 independent work units = 4 batches x 2 directions; one unit per
core (SPMD — same program, different inputs). Per unit: queries Q[8192,3]
vs keys K[8192,3].

Device algorithm per unit (segment-summary design):
  u[p,q] = 2*q_p.k_q - ||k_q||^2   (the per-query constant ||q||^2 drops
  out of the argmax and is reapplied on the host).

  * The matmul runs in bf16 at full PE rate with fp32-class accuracy via a
    3-way mantissa split (Dekker/Ozaki style): a = 2q and k are each split
    into 3 bf16 terms (8 mantissa bits per term); all cross products with
    i+j<=2 are kept. That is 6 index pairs x 3 dims = 18 contraction rows
    plus 3 rows for the bf16-split of -||k||^2 against constant-1 weights:
    one K=21 bf16 matmul per tile. Dropped terms are O(2^-27); PSUM
    accumulates fp32, so u matches the f32 reference to ~2e-6 abs — the
    same near-tie flip class as an fp32 matmul.
  * TensorE: K=21/M=128/N=512 matmuls fill [128,2048] 4-bank PSUM
    supertiles (two supertiles ping-pong).
  * VectorE: per supertile ONE segmented reduce (3D AP [128,32,64], axis X,
    op max) PSUM -> SBUF: the max of u over each 64-key segment. This is
    the only full-data pass outside the PE.
  * Sync engine (SP): stages weights and streams each block's segment
    maxima [128,128] to DRAM.

  The host then takes, per query, argmax over its 128 segment maxima
  (exact: the true argmax key lies in that segment BY CONSTRUCTION of the
  device's u values) and refines within the 64-key segment with the same
  f32 expansion formula the reference uses (1/128 of the device work).

The whole program runs inside hardware Fori loops (pair-of-blocks bodies so
buffers alternate with static APs). Cross-engine sync is credit-based:
semaphores + per-waiter credit registers, with `nop().then_inc(sem, n)`
priming standing in for negative initial credits.
"""

from contextlib import ExitStack

import numpy as np
import ml_dtypes

import concourse.bass as bass
import concourse.mybir as mybir
from concourse.bass import ds
from concourse.bass_utils import run_bass_kernel_spmd

F32 = mybir.dt.float32
BF16 = mybir.dt.bfloat16

N_BATCH = 4
NPTS = 8192
N_CORES = 8
KDIM = 21                        # 6 split-pairs x 3 dims + 3 norm rows
SEG = 64                         # keys per segment
# cross terms (i,j) of the 3-way bf16 split with i+j<=2
PAIRS = [(0, 0), (0, 1), (1, 0), (1, 1), (0, 2), (2, 0)]


def build_chamfer_bass(P1=NPTS, P2=NPTS, repeat=1):
    """Single-core Bass program (SPMD across cores)."""
    sup = 2048                       # keys per supertile (4 PSUM banks)
    assert P1 % 256 == 0 and P2 % (2 * sup) == 0
    qb = P1 // 128                   # query blocks
    pairs = qb // 2                  # 2 blocks per loop iteration
    nsup = P2 // sup                 # supertiles per block (even)
    ntile = sup // 512
    nseg = sup // SEG                # segments per supertile (32)
    segb = P2 // SEG                 # segments per block (128)
    assert nsup % 2 == 0

    nc = bass.Bass()
    qka = nc.dram_tensor("qka", [KDIM, P1 + P2], BF16, kind="ExternalInput")
    smax = nc.dram_tensor("smax", [128, qb * segb], F32,
                          kind="ExternalOutput")

    with ExitStack() as ctx:
        ec = ctx.enter_context
        keys_sb = ec(nc.sbuf_tensor([KDIM, P2], BF16))
        wstage = ec(nc.sbuf_tensor([KDIM, 256], BF16))  # A: 0:128, B: 128:
        smax_sb = ec(nc.sbuf_tensor([128, 2 * segb], F32))  # 2 block slots
        ps_a = ec(nc.psum_tensor([128, sup], F32))
        ps_b = ec(nc.psum_tensor([128, sup], F32))
        s_dma = ec(nc.semaphore("s_dma"))
        s_stage = ec(nc.semaphore("s_stage"))
        s_mm = ec(nc.semaphore("s_mm"))
        s_red = ec(nc.semaphore("s_red"))
        s_out = ec(nc.semaphore("s_out"))
        block = ec(nc.Block())
        ps = [ps_a, ps_b]

        @block.sync
        def _(sync):
            sync.dma_start(out=keys_sb[:, :], in_=qka[:, P1:P1 + P2]
                           ).then_inc(s_dma, 16)
            qoff = sync.alloc_register("qoff")
            breg = sync.alloc_register("breg")
            r_pe = sync.alloc_register("r_pe")
            r_sg = sync.alloc_register("r_sg")
            r_rd = sync.alloc_register("r_rd")
            sync.reg_mov(r_pe, 0)
            sync.reg_mov(r_sg, 0)
            # s_red is primed +2 by vector for psum reuse; account for it
            sync.reg_mov(r_rd, 2)

            def stage_half(half):
                # one 128-query weight block into wstage half A or B
                qv = sync.snap(qoff, min_val=0, max_val=P1)
                sync.dma_start(out=wstage[:, 128 * half:128 * half + 128],
                               in_=qka[:, ds(qv, 128)]).then_inc(s_stage, 16)
                sync.reg_add(qoff, qoff, 128)

            def outs_block(half):
                # all nsup reduces of this block must be done
                sync.reg_add(r_rd, r_rd, nsup)
                sync.wait_ge(s_red, r_rd)
                bv = sync.snap(breg, min_val=0, max_val=(qb - 1) * segb)
                sync.dma_start(out=smax[:, ds(bv, segb)],
                               in_=smax_sb[:, half * segb:(half + 1) * segb]
                               ).then_inc(s_out, 16)
                sync.reg_add(breg, breg, segb)

            assert pairs >= 2
            with sync.Fori(0, repeat, 1):
                sync.reg_mov(qoff, 0)
                sync.reg_mov(breg, 0)
                # previous rep's staging fully drained (wstage WAW)
                sync.wait_ge(s_stage, r_sg)
                sync.reg_add(r_sg, r_sg, 32 * pairs)
                stage_half(0)                     # pair 0 block A
                stage_half(1)                     # pair 0 block B
                with sync.Fori(0, pairs - 1, 1):
                    # refill each wstage half for pair i+1 as soon as the
                    # matching block of pair i has issued its matmuls, so
                    # the DMA hides under the other block's compute
                    sync.reg_add(r_pe, r_pe, nsup)
                    sync.wait_ge(s_mm, r_pe)      # pair i block A done
                    stage_half(0)
                    sync.reg_add(r_pe, r_pe, nsup)
                    sync.wait_ge(s_mm, r_pe)      # pair i block B done
                    stage_half(1)
                    outs_block(0)
                    outs_block(1)
                outs_block(0)                     # last pair's outputs
                outs_block(1)
                # account the last pair's matmuls (rep-boundary WAW)
                sync.reg_add(r_pe, r_pe, 2 * nsup)
                sync.wait_ge(s_mm, r_pe)
            sync.wait_ge(s_out, 16 * qb * repeat)
            sync.wait_ge(s_dma, 16)

        @block.tensor
        def _(tensor):
            tensor.wait_ge(s_dma, 16)
            r_stage = tensor.alloc_register("r_stage")
            r_red = tensor.alloc_register("r_red")
            tensor.reg_mov(r_stage, 0)
            tensor.reg_mov(r_red, 0)
            with tensor.Fori(0, repeat, 1):
                with tensor.Fori(0, pairs, 1):
                    for half in range(2):
                        # this half's weights staged (in-order DMA queue)
                        tensor.reg_add(r_stage, r_stage, 16)
                        tensor.wait_ge(s_stage, r_stage)
                        lhsT = wstage[:, 128 * half:128 * half + 128]
                        for s in range(nsup):
                            # psum slot free: reduce of its previous use
                            # done (s_red primed with 2 credits)
                            tensor.reg_add(r_red, r_red, 1)
                            tensor.wait_ge(s_red, r_red)
                            last = None
                            for t in range(ntile):
                                last = nc.tensor.matmul(
                                    ps[s % 2][:, 512 * t:512 * (t + 1)],
                                    lhsT,
                                    keys_sb[:, sup * s + 512 * t:
                                            sup * s + 512 * (t + 1)],
                                    start=True, stop=True,
                                )
                            last.then_inc(s_mm, 1)

        @block.vector
        def _(vector):
            vector.wait_ge(s_dma, 16)
            vector.nop().then_inc(s_red, 2)       # psum-reuse priming
            vector.nop().then_inc(s_out, 32)      # smax_sb slot priming
            r_mm = vector.alloc_register("r_mm")
            r_ow = vector.alloc_register("r_ow")
            vector.reg_mov(r_mm, 0)
            vector.reg_mov(r_ow, 0)
            with vector.Fori(0, pairs * repeat, 1):
                for half in range(2):
                    # smax_sb slot free: SP's copy-out of its previous
                    # occupant (block b-2) completed (s_out primed +32)
                    vector.reg_add(r_ow, r_ow, 16)
                    vector.wait_ge(s_out, r_ow)
                    out_slot = smax_sb[:, half * segb:(half + 1) * segb]
                    for s in range(nsup):
                        vector.reg_add(r_mm, r_mm, 1)
                        vector.wait_ge(s_mm, r_mm)
                        nc.vector.tensor_reduce(
                            out=out_slot[:, nseg * s:nseg * (s + 1)],
                            in_=ps[s % 2][:, :].rearrange(
                                "p (s w) -> p s w", w=SEG),
                            axis=mybir.AxisListType.X,
                            op=mybir.AluOpType.max,
                        ).then_inc(s_red, 1)

    return nc


def _split3(a32):
    """3-way bf16 mantissa split of a float32 array: a ~= s0+s1+s2."""
    s0 = a32.astype(ml_dtypes.bfloat16)
    r = a32 - s0.astype(np.float32)
    s1 = r.astype(ml_dtypes.bfloat16)
    r2 = r - s1.astype(np.float32)
    s2 = r2.astype(ml_dtypes.bfloat16)
    return s0, s1, s2


def make_unit_inputs(q, k):
    """Host-side augmentation for one (query cloud, key cloud) unit."""
    q = np.ascontiguousarray(q, np.float32)
    k = np.ascontiguousarray(k, np.float32)
    p1, p2 = q.shape[0], k.shape[0]
    asp = _split3(2.0 * q)               # each (p1, 3) bf16
    ksp = _split3(k)                     # each (p2, 3) bf16
    nsp = _split3(-(k * k).sum(-1, dtype=np.float32))  # each (p2,) bf16
    qka = np.zeros((KDIM, p1 + p2), ml_dtypes.bfloat16)
    for r, (i, j) in enumerate(PAIRS):
        qka[3 * r:3 * r + 3, :p1] = asp[i].T
        qka[3 * r:3 * r + 3, p1:] = ksp[j].T
    qka[18:21, :p1] = ml_dtypes.bfloat16(1.0)
    for m in range(3):
        qka[18 + m, p1:] = nsp[m]
    return {"qka": qka}


_BUILT = {}


def _built_nc():
    if "nc" not in _BUILT:
        _BUILT["nc"] = build_chamfer_bass()
    return _BUILT["nc"]


def _refine_unit(smax_unit, q, k):
    """Exact per-query argmin from the device's segment maxima.

    smax_unit: [128, qb*128] f32 — row = query-within-block, block-major
    columns of 128 segment maxima of u = 2q.k - ||k||^2 (max u = min d2
    up to the per-query constant ||q||^2)."""
    p1 = q.shape[0]
    qb = p1 // 128
    segb = smax_unit.shape[1] // qb
    # [row, block, seg] -> [query, seg]
    sm = smax_unit.reshape(128, qb, segb).transpose(1, 0, 2).reshape(
        p1, segb)
    seg = np.argmax(sm, axis=1)          # first occurrence, like argmin d2
    base = (seg * SEG).astype(np.int64)
    cols = base[:, None] + np.arange(SEG)[None, :]
    yy = (k * k).sum(-1, dtype=np.float32)
    # reference formula, f32: ||q||^2 + ||k||^2 - 2 q.k
    cross = np.einsum('pd,pcd->pc', q, k[cols], dtype=np.float32)
    d2 = ((q * q).sum(-1, dtype=np.float32)[:, None] + yy[cols]
          - 2.0 * cross)
    loc = np.argmin(d2, axis=1)
    idx = (base + loc).astype(np.int32)
    cham = d2[np.arange(p1), loc]
    return cham.astype(np.float32), idx


def kernel(x, y, _collect_results=None):
    """Full-input entry point. x, y: (4, 8192, 3) float32.

    Returns (cham_x, cham_y, idx_x, idx_y) matching reference()."""
    x = np.asarray(x, np.float32)
    y = np.asarray(y, np.float32)
    n = x.shape[0]
    units = []
    in_maps = []
    for b in range(n):
        for d in range(2):
            q, k = (x[b], y[b]) if d == 0 else (y[b], x[b])
            in_maps.append(make_unit_inputs(q, k))
            units.append((b, d))
    nc = _built_nc()
    res = run_bass_kernel_spmd(nc, in_maps, core_ids=list(range(N_CORES)))
    if _collect_results is not None:
        _collect_results.append(res)
    cham_x = np.empty((n, x.shape[1]), np.float32)
    cham_y = np.empty((n, y.shape[1]), np.float32)
    idx_x = np.empty((n, x.shape[1]), np.int32)
    idx_y = np.empty((n, y.shape[1]), np.int32)
    for (b, d), r in zip(units, res.results):
        q, k = (x[b], y[b]) if d == 0 else (y[b], x[b])
        cham, idx = _refine_unit(np.asarray(r["smax"]), q, k)
        if d == 0:
            cham_x[b], idx_x[b] = cham, idx
        else:
            cham_y[b], idx_y[b] = cham, idx
    return cham_x, cham_y, idx_x, idx_y
